# revision 7
# baseline (speedup 1.0000x reference)
"""Deformable attention layer on 8 Trainium2 NeuronCores (Bass/Tile).

Strategy
--------
Data-parallel over batch, but sharded by ``b mod 8``: the reference samples
image ``j = (8*b + h) % 1024`` for batch ``b`` / head ``h`` (torch-repeat
quirk), so batches ``b ≡ c (mod 8)`` touch exactly the 128 images
``{64*u + 8*c + h}`` — a disjoint per-core set.

The bilinear gather is the whole game.  The host pre-builds, per core, a
"row-pair" table ``fmt[(s, y, x), :] = [fm[s, y, x, :], fm[s, y+1, x, :]]``
(channels-last, 64 floats = 256 B per row).  One indirect-DMA descriptor
fetching 128 contiguous floats starting at row ``(s, ys, xs)`` then yields
the full 2x2x32 interpolation patch.  Out-of-bounds corners are handled by
remapping the bilinear weights onto the clipped patch (weights of corners
that fall outside the map become exactly zero), which reproduces
grid_sample's zero padding exactly.

Per core: a few tiny matmuls (q/offset/pos/vel projections), ~30 DVE ops of
coordinate math, 128 indirect gathers (one per sampled point, 128 batches
per gather), and per-head softmax-attention done entirely as per-partition
free-dim vector ops (partition dim = batch).
"""

import sys

sys.path.insert(0, "/opt/trn_rl_repo")

import numpy as np

import concourse.bass as bass
import concourse.tile as tile
from concourse import mybir
from concourse.bass_utils import run_bass_kernel_spmd
from concourse.masks import make_identity
from concourse.vector_clock import ScopedClock

F32 = mybir.dt.float32
I32 = mybir.dt.int32

B = 1024
IN = 256
OUT = 256
NH = 8
NPT = 16
DK = 32
FH = 128
FW = 128
NCORES = 8
BL = B // NCORES          # 128 local batches per core
HP = NH * NPT             # 128 sampled points per batch element
RROWS = BL * FH * FW      # 2097152 table rows per core


# ----------------------------------------------------------------------------
# Tail-drain fix: this walrus build rejects instructions carrying more than
# one sync wait; split the TileContext exit drain into single-wait drains.
# ----------------------------------------------------------------------------
def _patched_drain_and_barrier(self, tick_clock, wait_clock):
    drain_inst = self.nc.sync.drain()
    wait_clock.add_sem_waits(
        drain_inst.ins, ScopedClock({None: tick_clock.global_clock})
    )
    si = drain_inst.ins.sync_info
    if si is not None and len(si.on_wait) > 1:
        waits = list(si.on_wait)
        si.on_wait = waits[:1]
        drain_inst.ins.sync_info = si
        for w in waits[1:]:
            d2 = self.nc.sync.drain()
            d2.ins.sync_info = mybir.SyncInfo(on_wait=[w], on_update=[])
    self.nc.all_engine_barrier()
    popped = self.nc._tile_sem_poison_stack.pop()
    assert popped is self._sem_poison
    self.nc.clear_and_free_semaphores(list(self.sems.allocated().values()))
    self.nc.all_engine_barrier()


tile.TileContext._drain_and_barrier = _patched_drain_and_barrier

_SPLITW_N = [0]


def _split_multi_waits(nc):
    """This walrus build rejects instructions with more than one sync wait.

    Hoist all-but-one wait of every instruction onto single-wait NoOps
    inserted just before it on the same engine queue (same-engine program
    order makes the waits happen-before the instruction).
    """
    for fn in nc.m.functions:
        for bb in fn.blocks:
            out = []
            for inst in bb.instructions:
                si = inst.sync_info
                if si is not None and len(si.on_wait) > 1:
                    waits = list(si.on_wait)
                    for w in waits[:-1]:
                        _SPLITW_N[0] += 1
                        nop = mybir.InstNoOp(
                            name=f"I-splitw-{_SPLITW_N[0]}",
                            engine=inst.engine,
                            sync_info=mybir.SyncInfo(on_wait=[w], on_update=[]),
                            bass_nofuse=True,
                        )
                        out.append(nop)
                    si.on_wait = waits[-1:]
                    inst.sync_info = si
                out.append(inst)
            bb.instructions = out


def _bcast_part(ap, nparts):
    """Broadcast a [1, ...] DRAM AP across nparts partitions (step-0 dim)."""
    return bass.AP(tensor=ap.tensor, offset=ap.offset, ap=[[0, nparts]] + ap.ap[1:])


def _bcast_mid(ap, n):
    """Insert a broadcast (step-0, count-n) dim after the partition dim."""
    return bass.AP(
        tensor=ap.tensor, offset=ap.offset, ap=[ap.ap[0], [0, n]] + ap.ap[1:]
    )


def build_nc(debug=False, split_waits=True):
    nc = bass.Bass()

    fmt = nc.declare_dram_parameter("fmt", [RROWS, 64], F32, isOutput=False)
    xT = nc.declare_dram_parameter("xT", [IN, BL], F32, isOutput=False)
    wqT = nc.declare_dram_parameter("wqT", [IN, OUT], F32, isOutput=False)
    woffT = nc.declare_dram_parameter("woffT", [IN, 256], F32, isOutput=False)
    woutT = nc.declare_dram_parameter("woutT", [OUT, OUT], F32, isOutput=False)
    wpv = nc.declare_dram_parameter("wpv", [IN, 4], F32, isOutput=False)
    bq = nc.declare_dram_parameter("bq", [1, OUT], F32, isOutput=False)
    boff = nc.declare_dram_parameter("boff", [1, 256], F32, isOutput=False)
    bpv = nc.declare_dram_parameter("bpv", [1, 4], F32, isOutput=False)
    bout = nc.declare_dram_parameter("bout", [1, OUT], F32, isOutput=False)
    mbase = nc.declare_dram_parameter("mbase", [BL, 1], F32, isOutput=False)
    hbase = nc.declare_dram_parameter("hbase", [1, HP], F32, isOutput=False)

    out_d = nc.declare_dram_parameter("out", [BL, OUT], F32, isOutput=True)
    ref_d = nc.declare_dram_parameter("ref", [BL, 2], F32, isOutput=True)
    vel_d = nc.declare_dram_parameter("vel", [BL, 2], F32, isOutput=True)
    if debug:
        dbg_loc = nc.declare_dram_parameter("dbg_loc", [BL, HP, 2], F32, isOutput=True)
        dbg_flr = nc.declare_dram_parameter("dbg_flr", [BL, HP, 2], F32, isOutput=True)
        dbg_w4 = nc.declare_dram_parameter("dbg_w4", [BL, HP, 4], F32, isOutput=True)
        dbg_row = nc.declare_dram_parameter("dbg_row", [BL, HP], I32, isOutput=True)
        dbg_v = nc.declare_dram_parameter("dbg_v", [BL, NPT, DK], F32, isOutput=True)
        dbg_at = nc.declare_dram_parameter("dbg_at", [BL, NPT], F32, isOutput=True)

    with tile.TileContext(nc) as tc:
        with (
            tc.tile_pool(name="const", bufs=1) as const,
            tc.tile_pool(name="work", bufs=1) as work,
            tc.tile_pool(name="ps", bufs=1, space="PSUM") as ps,
            tc.tile_pool(name="spool", bufs=3) as spool,
        ):
            # ---------------- constants / weights -------------------------
            xT_t = const.tile([128, 2, BL], F32)
            nc.sync.dma_start(
                out=xT_t[:], in_=xT[:].rearrange("(t k) b -> k t b", k=128)
            )
            wq_t = const.tile([128, 2, OUT], F32)
            nc.sync.dma_start(
                out=wq_t[:], in_=wqT[:].rearrange("(t k) o -> k t o", k=128)
            )
            woff_t = const.tile([128, 2, 256], F32)
            nc.sync.dma_start(
                out=woff_t[:], in_=woffT[:].rearrange("(t k) o -> k t o", k=128)
            )
            wpv_t = const.tile([128, 2, 4], F32)
            nc.sync.dma_start(
                out=wpv_t[:], in_=wpv[:].rearrange("(t k) o -> k t o", k=128)
            )
            wout_t = const.tile([128, 2, OUT], F32)
            nc.sync.dma_start(
                out=wout_t[:], in_=woutT[:].rearrange("(t k) o -> k t o", k=128)
            )
            bq_t = const.tile([128, OUT], F32)
            nc.gpsimd.dma_start(out=bq_t[:], in_=_bcast_part(bq[:], 128))
            boff_t = const.tile([128, 256], F32)
            nc.gpsimd.dma_start(out=boff_t[:], in_=_bcast_part(boff[:], 128))
            bpv_t = const.tile([128, 4], F32)
            nc.gpsimd.dma_start(out=bpv_t[:], in_=_bcast_part(bpv[:], 128))
            bout_t = const.tile([128, OUT], F32)
            nc.gpsimd.dma_start(out=bout_t[:], in_=_bcast_part(bout[:], 128))
            hb_t = const.tile([128, HP], F32)
            nc.gpsimd.dma_start(out=hb_t[:], in_=_bcast_part(hbase[:], 128))
            mb_t = const.tile([BL, 1], F32)
            nc.sync.dma_start(out=mb_t[:], in_=mbase[:])
            ident = const.tile([128, 128], F32)
            make_identity(nc, ident[:])

            # ---------------- dense projections ---------------------------
            q_ps = ps.tile([128, OUT], F32)
            off_ps = ps.tile([128, 256], F32)
            pv_ps = ps.tile([128, 4], F32)
            for t in range(2):
                nc.tensor.matmul(
                    out=q_ps[:], lhsT=xT_t[:, t, :], rhs=wq_t[:, t, :],
                    start=(t == 0), stop=(t == 1),
                )
            for t in range(2):
                nc.tensor.matmul(
                    out=off_ps[:], lhsT=xT_t[:, t, :], rhs=woff_t[:, t, :],
                    start=(t == 0), stop=(t == 1),
                )
            for t in range(2):
                nc.tensor.matmul(
                    out=pv_ps[:], lhsT=xT_t[:, t, :], rhs=wpv_t[:, t, :],
                    start=(t == 0), stop=(t == 1),
                )
            q_sb = work.tile([128, OUT], F32)
            nc.vector.tensor_add(q_sb[:], q_ps[:], bq_t[:])
            off_sb = work.tile([128, 256], F32)
            nc.vector.tensor_add(off_sb[:], off_ps[:], boff_t[:])
            pv_sb = work.tile([128, 4], F32)
            nc.vector.tensor_add(pv_sb[:], pv_ps[:], bpv_t[:])
            nc.sync.dma_start(out=ref_d[:], in_=pv_sb[:, 0:2])
            nc.sync.dma_start(out=vel_d[:], in_=pv_sb[:, 2:4])

            # ---------------- sampling coordinates ------------------------
            # x and y are processed together, interleaved as (hp, 2).
            loc = work.tile([128, HP, 2], F32)
            nc.vector.tensor_tensor(
                out=loc[:],
                in0=off_sb[:].rearrange("b (hp two) -> b hp two", two=2),
                in1=_bcast_mid(pv_sb[:, 0:2], HP),
                op=mybir.AluOpType.add,
            )
            if debug:
                nc.sync.dma_start(out=dbg_loc[:], in_=loc[:])
            # ix = ((g + 1) * 128 - 1) * 0.5, with the reference's rounding
            t1 = work.tile([128, HP, 2], F32)
            nc.vector.tensor_scalar(
                out=t1[:], in0=loc[:], scalar1=1.0, scalar2=float(FW),
                op0=mybir.AluOpType.add, op1=mybir.AluOpType.mult,
            )
            ixy = work.tile([128, HP, 2], F32)
            nc.vector.tensor_scalar(
                out=ixy[:], in0=t1[:], scalar1=1.0, scalar2=0.5,
                op0=mybir.AluOpType.subtract, op1=mybir.AluOpType.mult,
            )
            # floor() robust to the cast's rounding mode
            fi = work.tile([128, HP, 2], I32)
            nc.vector.tensor_copy(fi[:], ixy[:])
            ff = work.tile([128, HP, 2], F32)
            nc.vector.tensor_copy(ff[:], fi[:])
            gt = work.tile([128, HP, 2], F32)
            nc.vector.tensor_tensor(
                out=gt[:], in0=ff[:], in1=ixy[:], op=mybir.AluOpType.is_gt
            )
            flr = work.tile([128, HP, 2], F32)
            nc.vector.tensor_tensor(
                out=flr[:], in0=ff[:], in1=gt[:], op=mybir.AluOpType.subtract
            )
            if debug:
                nc.sync.dma_start(out=dbg_flr[:], in_=flr[:])
            w1 = work.tile([128, HP, 2], F32)
            nc.vector.tensor_tensor(
                out=w1[:], in0=ixy[:], in1=flr[:], op=mybir.AluOpType.subtract
            )
            w0 = work.tile([128, HP, 2], F32)
            nc.vector.tensor_scalar(
                out=w0[:], in0=w1[:], scalar1=1.0, scalar2=-1.0,
                op0=mybir.AluOpType.subtract, op1=mybir.AluOpType.mult,
            )
            # clipped patch start, and weight remap onto the fetched 2x2 patch
            cs = work.tile([128, HP, 2], F32)
            nc.vector.tensor_scalar(
                out=cs[:], in0=flr[:], scalar1=0.0, scalar2=float(FW - 2),
                op0=mybir.AluOpType.max, op1=mybir.AluOpType.min,
            )
            csm = work.tile([128, HP, 2], F32)
            nc.vector.tensor_scalar(
                out=csm[:], in0=cs[:], scalar1=1.0, scalar2=None,
                op0=mybir.AluOpType.subtract,
            )
            csp = work.tile([128, HP, 2], F32)
            nc.vector.tensor_scalar(
                out=csp[:], in0=cs[:], scalar1=1.0, scalar2=None,
                op0=mybir.AluOpType.add,
            )
            eq0 = work.tile([128, HP, 2], F32)
            nc.vector.tensor_tensor(
                out=eq0[:], in0=flr[:], in1=cs[:], op=mybir.AluOpType.is_equal
            )
            eqm = work.tile([128, HP, 2], F32)
            nc.vector.tensor_tensor(
                out=eqm[:], in0=flr[:], in1=csm[:], op=mybir.AluOpType.is_equal
            )
            eqp = work.tile([128, HP, 2], F32)
            nc.vector.tensor_tensor(
                out=eqp[:], in0=flr[:], in1=csp[:], op=mybir.AluOpType.is_equal
            )
            # u0 = w0*[f==cs] + w1*[f==cs-1] ; u1 = w0*[f==cs+1] + w1*[f==cs]
            m1 = work.tile([128, HP, 2], F32)
            nc.vector.tensor_mul(m1[:], w0[:], eq0[:])
            m2 = work.tile([128, HP, 2], F32)
            nc.vector.tensor_mul(m2[:], w1[:], eqm[:])
            u0 = work.tile([128, HP, 2], F32)
            nc.vector.tensor_add(u0[:], m1[:], m2[:])
            m3 = work.tile([128, HP, 2], F32)
            nc.vector.tensor_mul(m3[:], w0[:], eqp[:])
            m4 = work.tile([128, HP, 2], F32)
            nc.vector.tensor_mul(m4[:], w1[:], eq0[:])
            u1 = work.tile([128, HP, 2], F32)
            nc.vector.tensor_add(u1[:], m3[:], m4[:])
            # per-corner weights, slot order (xslot, yslot)
            w4 = work.tile([128, HP, 4], F32)
            nc.vector.tensor_mul(w4[:, :, 0], u0[:, :, 0], u0[:, :, 1])
            nc.vector.tensor_mul(w4[:, :, 1], u0[:, :, 0], u1[:, :, 1])
            nc.vector.tensor_mul(w4[:, :, 2], u1[:, :, 0], u0[:, :, 1])
            nc.vector.tensor_mul(w4[:, :, 3], u1[:, :, 0], u1[:, :, 1])
            if debug:
                nc.sync.dma_start(out=dbg_w4[:], in_=w4[:])
            # table row = mbase[m] + 16384*h + 128*ys + xs
            r1 = work.tile([128, HP], F32)
            nc.vector.tensor_scalar(
                out=r1[:], in0=cs[:, :, 1], scalar1=float(FW), scalar2=None,
                op0=mybir.AluOpType.mult,
            )
            r2 = work.tile([128, HP], F32)
            nc.vector.tensor_tensor(
                out=r2[:], in0=r1[:], in1=cs[:, :, 0], op=mybir.AluOpType.add
            )
            r3 = work.tile([128, HP], F32)
            nc.vector.tensor_tensor(
                out=r3[:], in0=r2[:], in1=hb_t[:], op=mybir.AluOpType.add
            )
            r4 = work.tile([128, HP], F32)
            nc.vector.tensor_tensor(
                out=r4[:], in0=r3[:], in1=mb_t[:, 0:1].to_broadcast([128, HP]),
                op=mybir.AluOpType.add,
            )
            rowi = work.tile([128, HP], I32)
            nc.vector.tensor_copy(rowi[:], r4[:])
            if debug:
                nc.sync.dma_start(out=dbg_row[:], in_=rowi[:])

            # ---------------- gather + attention, one head at a time ------
            outh = work.tile([128, OUT], F32)
            inv_sqrt_dk = float(np.float32(1.0 / np.sqrt(np.float64(DK))))
            for h in range(NH):
                sh = spool.tile([128, NPT, 128], F32, tag="sh")
                for p in range(NPT):
                    g = h * NPT + p
                    nc.gpsimd.indirect_dma_start(
                        out=sh[:, p, :],
                        out_offset=None,
                        in_=fmt[:],
                        in_offset=bass.IndirectOffsetOnAxis(
                            ap=rowi[:, g : g + 1], axis=0
                        ),
                    )
                # patch layout per point: (xslot 2, yslot 2, d 32) = (s 4, d 32)
                wz = spool.tile([128, NPT, 4, DK], F32, tag="wz")
                nc.vector.tensor_tensor(
                    out=wz[:],
                    in0=sh[:].rearrange("b p (s d) -> b p s d", s=4),
                    in1=w4[:, h * NPT : (h + 1) * NPT, :].to_broadcast(
                        [128, NPT, 4, DK]
                    ),
                    op=mybir.AluOpType.mult,
                )
                v = spool.tile([128, NPT, DK], F32, tag="v")
                nc.vector.tensor_reduce(
                    out=v[:],
                    in_=wz[:].rearrange("b p s d -> b p d s"),
                    axis=mybir.AxisListType.X,
                    op=mybir.AluOpType.add,
                )
                if debug and h == 0:
                    nc.sync.dma_start(out=dbg_v[:], in_=v[:])
                qs = spool.tile([128, NPT, DK], F32, tag="qs")
                nc.vector.tensor_tensor(
                    out=qs[:],
                    in0=v[:],
                    in1=_bcast_mid(q_sb[:, h * DK : (h + 1) * DK], NPT),
                    op=mybir.AluOpType.mult,
                )
                sc = spool.tile([128, NPT], F32, tag="sc")
                nc.vector.tensor_reduce(
                    out=sc[:], in_=qs[:], axis=mybir.AxisListType.X,
                    op=mybir.AluOpType.add,
                )
                scs = spool.tile([128, NPT], F32, tag="scs")
                nc.vector.tensor_scalar(
                    out=scs[:], in0=sc[:], scalar1=inv_sqrt_dk, scalar2=None,
                    op0=mybir.AluOpType.mult,
                )
                mx = spool.tile([128, 1], F32, tag="mx")
                nc.vector.tensor_reduce(
                    out=mx[:], in_=scs[:], axis=mybir.AxisListType.X,
                    op=mybir.AluOpType.max,
                )
                sub = spool.tile([128, NPT], F32, tag="sub")
                nc.vector.tensor_tensor(
                    out=sub[:], in0=scs[:], in1=mx[:, 0:1].to_broadcast([128, NPT]),
                    op=mybir.AluOpType.subtract,
                )
                ex = spool.tile([128, NPT], F32, tag="ex")
                nc.scalar.activation(
                    out=ex[:], in_=sub[:], func=mybir.ActivationFunctionType.Exp
                )
                sm = spool.tile([128, 1], F32, tag="sm")
                nc.vector.tensor_reduce(
                    out=sm[:], in_=ex[:], axis=mybir.AxisListType.X,
                    op=mybir.AluOpType.add,
                )
                rec = spool.tile([128, 1], F32, tag="rec")
                nc.vector.reciprocal(rec[:], sm[:])
                at = spool.tile([128, NPT], F32, tag="at")
                nc.vector.tensor_tensor(
                    out=at[:], in0=ex[:], in1=rec[:, 0:1].to_broadcast([128, NPT]),
                    op=mybir.AluOpType.mult,
                )
                if debug and h == 0:
                    nc.sync.dma_start(out=dbg_at[:], in_=at[:])
                wz2 = spool.tile([128, NPT, DK], F32, tag="wz2")
                nc.vector.tensor_tensor(
                    out=wz2[:],
                    in0=v[:],
                    in1=bass.AP(
                        tensor=at[:].tensor,
                        offset=at[:].offset,
                        ap=at[:].ap + [[0, DK]],
                    ),
                    op=mybir.AluOpType.mult,
                )
                nc.vector.tensor_reduce(
                    out=outh[:, h * DK : (h + 1) * DK],
                    in_=wz2[:].rearrange("b p d -> b d p"),
                    axis=mybir.AxisListType.X,
                    op=mybir.AluOpType.add,
                )

            # ---------------- output projection ---------------------------
            ohT = work.tile([128, 2, 128], F32)
            for t in range(2):
                tp = ps.tile([128, 128], F32, tag="tp")
                nc.tensor.transpose(
                    tp[:], outh[:, t * 128 : (t + 1) * 128], ident[:]
                )
                nc.vector.tensor_copy(ohT[:, t, :], tp[:])
            o_ps = ps.tile([128, OUT], F32)
            for t in range(2):
                nc.tensor.matmul(
                    out=o_ps[:], lhsT=ohT[:, t, :], rhs=wout_t[:, t, :],
                    start=(t == 0), stop=(t == 1),
                )
            outf = work.tile([128, OUT], F32)
            nc.vector.tensor_add(outf[:], o_ps[:], bout_t[:])
            nc.sync.dma_start(out=out_d[:], in_=outf[:])

    if split_waits:
        _split_multi_waits(nc)
    return nc


# ----------------------------------------------------------------------------
# Host side
# ----------------------------------------------------------------------------
def make_core_inputs(x, feature_map, Wq, bq, Woff, boff, Wpos, bpos, Wvel,
                     bvel, Wout, bout, core):
    """Build the input map for one core (shard by b mod 8)."""
    c = core
    xs = np.ascontiguousarray(x[c::NCORES])            # (128, 256)
    ins = {
        "xT": np.ascontiguousarray(xs.T),
        "wqT": np.ascontiguousarray(Wq.T),
        "woffT": np.ascontiguousarray(Woff.T),
        "woutT": np.ascontiguousarray(Wout.T),
        "wpv": np.ascontiguousarray(
            np.concatenate([Wpos.T, Wvel.T], axis=1)
        ),
        "bq": bq.reshape(1, -1).astype(np.float32),
        "boff": boff.reshape(1, -1).astype(np.float32),
        "bpv": np.concatenate([bpos, bvel]).reshape(1, 4).astype(np.float32),
        "bout": bout.reshape(1, -1).astype(np.float32),
        "mbase": (131072.0 * (np.arange(BL) % 16)).astype(np.float32).reshape(BL, 1),
        "hbase": (16384.0 * (np.arange(HP) // NPT)).astype(np.float32).reshape(1, HP),
        "fmt": make_core_table(feature_map, c),
    }
    return ins


def make_core_table(feature_map, core):
    """Row-pair gather table for one core's 128 images."""
    c = core
    js = np.array(
        [64 * u + 8 * c + h for u in range(16) for h in range(NH)], dtype=np.int64
    )
    sub = feature_map[js]                              # (128, 32, 128, 128)
    subT = sub.transpose(0, 2, 3, 1)                   # (128, y, x, 32) view
    tbl = np.empty((BL, FH, FW, 64), dtype=np.float32)
    tbl[..., :DK] = subT
    tbl[:, : FH - 1, :, DK:] = subT[:, 1:]
    tbl[:, FH - 1, :, DK:] = subT[:, FH - 1]           # never addressed
    return tbl.reshape(RROWS, 64)


_NC_CACHE = {}


def get_nc(debug=False):
    if debug not in _NC_CACHE:
        _NC_CACHE[debug] = build_nc(debug)
    return _NC_CACHE[debug]


def assemble(results):
    output = np.empty((B, OUT), dtype=np.float32)
    ref = np.empty((B, 2), dtype=np.float32)
    vel = np.empty((B, 2), dtype=np.float32)
    for c, r in enumerate(results):
        output[c::NCORES] = r["out"]
        ref[c::NCORES] = r["ref"]
        vel[c::NCORES] = r["vel"]
    return output, ref, vel


def kernel(**inputs):
    inputs = {k: np.asarray(v) for k, v in inputs.items()}
    nc = get_nc()
    in_maps = [
        make_core_inputs(core=c, **inputs) for c in range(NCORES)
    ]
    res = run_bass_kernel_spmd(nc, in_maps, list(range(NCORES)))
    return assemble(res.results)
